# revision 14
# baseline (speedup 1.0000x reference)
"""RNN-T Joiner kernel for Trainium2 (Bass/Tile), 8-core data-parallel over batch.

out[b,t,u,v] = (enc[b,t] @ We)[v] + (pred[b,u] @ Wp)[v] + bias[v]

Layout trick: put V on SBUF partitions. Then for a fixed u, the pred term is a
per-partition scalar, so the broadcast-add is a DVE tensor_scalar_add (2x_1p
fp16 mode) or an Act-engine activation with per-partition bias — no PE one-hot
broadcast and no fp32 tensor_tensor adds. The fp16 datapath halves HBM store
traffic (34 MB/core vs 68 MB), which is the roofline term. Tolerance is 2e-2
rel; fp16 rounding contributes ~6e-4.

Per core (one batch element):
  - Inputs are host-pretiled so each tensor is ONE dma_start with multi-KB
    contiguous runs per partition (128x512B-descriptor loads were taking
    ~20 us before).
  - PE (fp16): enc_projT [v,t] and pred_projT [v,u] with V on output
    partitions; bias folded into pred_projT via a ones-row matmul.
  - Act: PSUM->SBUF evacuation + 4/13 of the per-u adds.
  - DVE: 9/13 of the per-u adds (tensor_scalar_add, fp32 per-partition
    scalar operand is exempt from the 2-byte packing rule).
  - HWDGE DMA: stores with HBM layout [v_lo, u, v_chunk, t] so each store
    half-block has ~25-29 KB contiguous runs; un-permuted on the host.
"""

import sys

sys.path.insert(0, "/opt/trn_rl_repo")

import numpy as np

B, T, U1, D, V = 8, 256, 65, 640, 1024
KC = D // 128   # 5 contraction chunks
VC = V // 128   # 8 vocab chunks
NU = 13         # u's per store block: 5 blocks x 13 = 65
NBLK = U1 // NU
NACT = 4        # of the NU u's per (vc, blk), how many go to Act

_COMPILED = None


def _build():
    import concourse.bacc as bacc
    import concourse.tile as tile
    import concourse.mybir as mybir

    f16 = mybir.dt.float16
    f32 = mybir.dt.float32

    nc = bacc.Bacc("TRN2", target_bir_lowering=False, debug=False, num_devices=8)

    # host-pretiled: encT[p, c, t] = enc.T[c*128+p, t], etc.
    encT = nc.dram_tensor("encT", [128, KC, T], f16, kind="ExternalInput")
    predT = nc.dram_tensor("predT", [128, KC, U1], f16, kind="ExternalInput")
    We = nc.dram_tensor("We", [128, KC, V], f16, kind="ExternalInput")
    Wp = nc.dram_tensor("Wp", [128, KC, V], f16, kind="ExternalInput")
    bias = nc.dram_tensor("bias", [1, V], f16, kind="ExternalInput")
    ones = nc.dram_tensor("ones", [1, U1], f16, kind="ExternalInput")
    # out[v_lo, u, v_chunk, t] ; v = v_chunk*128 + v_lo
    out = nc.dram_tensor("out", [128, U1, VC, T], f16, kind="ExternalOutput")

    with tile.TileContext(nc) as tc:
        with tc.tile_pool(name="consts", bufs=1) as cp:
            # load order = dependency order of the projection matmuls;
            # We is split so the first enc matmuls start before the rest lands
            encT_sb = cp.tile([128, KC, T], f16, tag="encT")
            nc.sync.dma_start(encT_sb[:], encT[:])
            We_sb = cp.tile([128, KC, V], f16, tag="We")
            nc.sync.dma_start(We_sb[:, 0:2, :], We[:, 0:2, :])
            nc.sync.dma_start(We_sb[:, 2:KC, :], We[:, 2:KC, :])
            Wp_sb = cp.tile([128, KC, V], f16, tag="Wp")
            nc.sync.dma_start(Wp_sb[:, 0:2, :], Wp[:, 0:2, :])
            nc.sync.dma_start(Wp_sb[:, 2:KC, :], Wp[:, 2:KC, :])
            predT_sb = cp.tile([128, KC, U1], f16, tag="predT")
            nc.sync.dma_start(predT_sb[:], predT[:])
            bias_sb = cp.tile([1, V], f16, tag="bias")
            nc.sync.dma_start(bias_sb[:], bias[:])
            ones_sb = cp.tile([1, U1], f16, tag="ones")
            nc.sync.dma_start(ones_sb[:], ones[:])

            encP = cp.tile([128, VC * T], f16, tag="encP")      # enc_projT[v, t]
            predP = cp.tile([128, VC * U1], f32, tag="predP")   # pred_projT[v, u] + b[v]

            # ---- projections: V on output partitions ----
            # All enc projections first (We arrives before Wp), then pred.
            with tc.tile_pool(name="ppool", bufs=2, space="PSUM") as pp:
                for vc in range(VC):
                    vs = slice(vc * 128, (vc + 1) * 128)
                    pse = pp.tile([128, T], f32, tag="pse")
                    for c in range(KC):
                        nc.tensor.matmul(
                            pse[:], We_sb[:, c, vs], encT_sb[:, c, :],
                            start=(c == 0), stop=(c == KC - 1))
                    nc.scalar.copy(encP[:, vc * T:(vc + 1) * T], pse[:])
                for vc in range(VC):
                    vs = slice(vc * 128, (vc + 1) * 128)
                    psp = pp.tile([128, U1], f32, tag="psp")
                    for c in range(KC):
                        nc.tensor.matmul(
                            psp[:], Wp_sb[:, c, vs], predT_sb[:, c, :],
                            start=(c == 0), stop=False)
                    nc.tensor.matmul(
                        psp[:], bias_sb[0:1, vs], ones_sb[0:1, :],
                        start=False, stop=True)
                    nc.scalar.copy(predP[:, vc * U1:(vc + 1) * U1], psp[:])

            # ---- main loop: per-u scalar-add, big interleaved stores ----
            # Each block is stored as two u-halves so the DMA starts after
            # roughly half the block's adds. Per u, the add goes to DVE
            # (tensor_scalar_add) or the Act engine (Identity + bias),
            # interleaved so both engines fill each half concurrently.
            with tc.tile_pool(name="outp", bufs=3) as op_:
                for blk in range(NBLK):
                    u0 = blk * NU
                    stage = op_.tile([128, NU, VC, T], f16, tag="stage")
                    if blk == 0:
                        splits = ((0, 3), (3, 7), (7, 10), (10, NU))
                    elif blk == NBLK - 1:
                        splits = ((0, 4), (4, 7), (7, 10), (10, NU))
                    else:
                        splits = ((0, 7), (7, NU))
                    for lo, hi in splits:
                        for ui in range(lo, hi):
                            u = u0 + ui
                            for vc in range(VC):
                                enc_ap = encP[:, vc * T:(vc + 1) * T]
                                sc_ap = predP[:, vc * U1 + u:vc * U1 + u + 1]
                                # alternate engines by (ui*VC+vc) parity-ish:
                                if (ui * VC + vc) % NU >= NU - NACT:
                                    nc.scalar.add(
                                        stage[:, ui, vc, :], enc_ap, sc_ap)
                                else:
                                    nc.vector.tensor_scalar_add(
                                        stage[:, ui, vc, :], enc_ap, sc_ap)
                        nc.sync.dma_start(
                            out[:, u0 + lo:u0 + hi, :, :],
                            stage[:, lo:hi, :, :])

    nc.compile()
    return nc


def _get_compiled():
    global _COMPILED
    if _COMPILED is None:
        _COMPILED = _build()
    return _COMPILED


def _in_maps(encoder_out, predictor_out, W, b):
    Wt = np.asarray(W, dtype=np.float16).reshape(2 * KC, 128, V)
    We = np.ascontiguousarray(Wt[:KC].transpose(1, 0, 2))
    Wp = np.ascontiguousarray(Wt[KC:].transpose(1, 0, 2))
    bias = np.ascontiguousarray(np.asarray(b, dtype=np.float16).reshape(1, V))
    ones = np.ones((1, U1), dtype=np.float16)
    maps = []
    for i in range(B):
        et = np.asarray(encoder_out[i], dtype=np.float16).T  # [D, T]
        pt = np.asarray(predictor_out[i], dtype=np.float16).T  # [D, U1]
        maps.append({
            "encT": np.ascontiguousarray(
                et.reshape(KC, 128, T).transpose(1, 0, 2)),
            "predT": np.ascontiguousarray(
                pt.reshape(KC, 128, U1).transpose(1, 0, 2)),
            "We": We,
            "Wp": Wp,
            "bias": bias,
            "ones": ones,
        })
    return maps


def run(encoder_out, predictor_out, W, b, trace=False, tmpdir=None):
    from concourse.bass_utils import run_bass_kernel_spmd

    nc = _get_compiled()
    maps = _in_maps(encoder_out, predictor_out, W, b)
    res = run_bass_kernel_spmd(
        nc, maps, list(range(B)), trace=trace,
        **({"tmpdir": tmpdir} if tmpdir else {}))
    outs = np.empty((B, T, U1, V), dtype=np.float32)
    for i in range(B):
        arr = res.results[i]["out"]  # [128, U1, VC, T] fp16
        outs[i] = arr.transpose(3, 1, 2, 0).reshape(T, U1, V).astype(np.float32)
    return outs, res


def kernel(encoder_out, predictor_out, W, b):
    outs, _ = run(encoder_out, predictor_out, W, b)
    return outs
